# revision 16
# baseline (speedup 1.0000x reference)
"""Causal GQA self-attention (B=2, S=2048, HID=2048, 16 q heads / 4 kv heads,
DH=128, interleaved RoPE) as a Trainium2 Bass/Tile kernel on 8 NeuronCores.

Sharding: core c -> (batch b = c // 4, kv-group g = c % 4). Each core computes
its batch's attention for the 4 q heads served by kv head g, plus the partial
output projection against Wo[:, group cols]; the host sums the 4 partials per
batch.

Device dataflow (per core), everything in "transposed" [feature, seq] layout:
  xT   [hid, s]   (host-pretransposed, bf16)
  qT   = WqT.T @ xT          (per head, psum f32)  -> rope -> bf16
  kT   = WkT.T @ xT          -> rope -> bf16
  v    [s, dh]    = xT.T @ WvT   (natural layout, bf16)
  sT   [s_k, s_q] = kT_tile.T @ qT_chunk      (one MM per tile, causal skip;
        the diagonal 512x512 block runs k-major with shrinking free dim)
  pT   = exp(sT * scale) (* diag mask)        (ACT, bf16; softmax max-free)
  den  = elementwise-accumulated pT on Pool/DVE (f32 sbuf), then one
         ones-matmul per (head, chunk) broadcasts the partition reduction
  oT  += v_tile.T @ pT                         (psum f32 accum)
  OT   = oT * (1/den)    (bf16)
  y   += OT_tile.T @ WoT_chunk  over heads    (psum f32) -> DRAM f32
"""

import os
import sys
from contextlib import ExitStack

import numpy as np

sys.path.insert(0, "/opt/trn_rl_repo")

import ml_dtypes

import concourse.bass as bass
import concourse.mybir as mybir
import concourse.tile as tile
from concourse import bacc
from concourse.bass_utils import run_bass_kernel_spmd

BF16 = mybir.dt.bfloat16
F32 = mybir.dt.float32
NP_BF16 = ml_dtypes.bfloat16

# problem constants
B, S, HID = 2, 2048, 2048
H, HK, DH = 16, 4, 128
NH = H // HK  # local q heads per core (= REP)
BASE = 10000.0
SCALE = 1.0 / float(np.sqrt(DH))

N_CORES = 8
QC = 512            # q-chunk (psum free dim)
KT = 128            # k-tile (partition dim)

LAST_RUN = {}


def build_nc(S=S, HID=HID, NH=NH):
    """Build the per-core Bass program. All cores run the same program (SPMD).

    Software pipeline: chunk c's section = [deferred finalize of c-1's last
    head] -> Q proj c -> attention c. K/V projections for chunk c+1 are
    interleaved INTO attention c as PE filler (they only need xq_{c+1} +
    weights), so the PE never stalls on the exp/mask chains; output
    projection pieces of chunk c-1 fill the remaining slack.
    """
    HT = HID // 128        # hid tiles
    SC = S // QC           # s chunks
    ST = S // 128          # s tiles
    TPC = QC // 128        # 128-tiles per chunk

    nc = bacc.Bacc()

    d_xt = nc.declare_dram_parameter("xt", [128, SC * HT * QC], BF16, isOutput=False)
    d_wq = nc.declare_dram_parameter("wq", [128, HT * 128 * NH], BF16, isOutput=False)
    d_wk = nc.declare_dram_parameter("wk", [128, HT * 128], BF16, isOutput=False)
    d_wv = nc.declare_dram_parameter("wv", [128, HT * 128], BF16, isOutput=False)
    d_wo = nc.declare_dram_parameter("wo", [128, NH * HID], BF16, isOutput=False)
    d_cos = nc.declare_dram_parameter("cos", [128, S], BF16, isOutput=False)
    d_sin = nc.declare_dram_parameter("sin", [128, S], BF16, isOutput=False)
    d_tri = nc.declare_dram_parameter("tri", [128, 128], BF16, isOutput=False)
    d_rt = nc.declare_dram_parameter("rt", [128, 128], BF16, isOutput=False)
    d_ones = nc.declare_dram_parameter("ones", [128, 128], BF16, isOutput=False)
    d_y = nc.declare_dram_parameter("y", [S, HID], BF16, isOutput=True)

    with tile.TileContext(nc) as tc, ExitStack() as ctx:
        const = ctx.enter_context(tc.tile_pool(name="const", bufs=1))
        xtp = ctx.enter_context(tc.tile_pool(name="xtp", bufs=2))
        work = ctx.enter_context(tc.tile_pool(name="work", bufs=1))
        ptp = ctx.enter_context(tc.tile_pool(name="ptp", bufs=8))
        ysp = ctx.enter_context(tc.tile_pool(name="ysp", bufs=4))
        psum = ctx.enter_context(tc.tile_pool(name="psum", bufs=1, space="PSUM"))

        # --- PE warmup: tiny matmuls on a memset tile keep the PE busy (and
        # the HAM clock un-throttled) while the first input DMAs land.
        warm = const.tile([128, 128], BF16, tag="warm")
        nc.vector.memset(warm[:, :], 0.0)
        for _ in range(16):
            wps = psum.tile([128, QC], F32, tag="dps", bufs=1)
            nc.tensor.matmul(wps[:, :128], warm[:, :], warm[:, :],
                             start=True, stop=True)

        # --- persistent constants (DMA order = need order: K-proj inputs
        # first, Wo (used only by the output projection) last) ---
        xq0 = xtp.tile([128, HT * QC], BF16, tag="xq")
        NDMA = 4  # chunk-load split for prefetches (issue cost ~0.7us each)
        W = HT * QC // NDMA
        W0 = W // 2  # chunk 0 streams in 8 finer pieces to pace the startup
        for i in range(2 * NDMA):
            nc.sync.dma_start(
                xq0[:, i * W0:(i + 1) * W0], d_xt[:, i * W0:(i + 1) * W0]
            )
        wk = const.tile([128, HT * 128], BF16, tag="wk")
        for i in range(4):
            nc.scalar.dma_start(
                wk[:, i * 512:(i + 1) * 512], d_wk[:, i * 512:(i + 1) * 512]
            )
        wq = const.tile([128, HT * 128 * NH], BF16, tag="wq")
        for j in range(2):  # heads 0-1 feed the interleaved startup pass
            nc.scalar.dma_start(
                wq[:, j * HT * 128:(j + 1) * HT * 128],
                d_wq[:, j * HT * 128:(j + 1) * HT * 128],
            )
        cos = const.tile([128, S], BF16, tag="cos")
        sin = const.tile([128, S], BF16, tag="sin")
        nc.scalar.dma_start(cos[:, :QC], d_cos[:, :QC])
        nc.scalar.dma_start(sin[:, :QC], d_sin[:, :QC])
        rt = const.tile([128, 128], BF16, tag="rt")
        nc.scalar.dma_start(rt[:, :], d_rt[:, :])
        wv = const.tile([128, HT * 128], BF16, tag="wv")
        nc.scalar.dma_start(wv[:, :], d_wv[:, :])
        for j in range(2, NH):
            nc.scalar.dma_start(
                wq[:, j * HT * 128:(j + 1) * HT * 128],
                d_wq[:, j * HT * 128:(j + 1) * HT * 128],
            )
        nc.scalar.dma_start(cos[:, QC:], d_cos[:, QC:])
        nc.scalar.dma_start(sin[:, QC:], d_sin[:, QC:])
        tri = const.tile([128, 128], BF16, tag="tri")
        nc.scalar.dma_start(tri[:, :], d_tri[:, :])
        ones = const.tile([128, 128], BF16, tag="ones")
        nc.scalar.dma_start(ones[:, :], d_ones[:, :])
        wo = const.tile([128, NH * HID], BF16, tag="wo")
        nc.scalar.dma_start(wo[:, :], d_wo[:, :])

        # persistent activations
        q_ro = const.tile([128, NH * S], BF16, tag="q_ro")
        k_ro = const.tile([128, S], BF16, tag="k_ro")
        v_nat = const.tile([128, S], BF16, tag="v_nat")
        ot = const.tile([128, NH * S], BF16, tag="ot")

        xq_tiles = {0: xq0}

        def rope(raw, out_slice, c):
            """raw: [128, QC] bf16 sbuf tile (pre-rope head block, dh on
            partitions). Writes bf16 roped output to out_slice."""
            rq = psum.tile([128, QC], F32, tag="sps", bufs=3)
            nc.tensor.matmul(rq[:, :], rt[:, :], raw[:, :], start=True, stop=True)
            t1 = work.tile([128, QC], BF16, tag="t1", bufs=2)
            nc.vector.tensor_tensor(
                t1[:, :], raw[:, :], cos[:, c * QC:(c + 1) * QC], mybir.AluOpType.mult
            )
            t2 = work.tile([128, QC], BF16, tag="t2", bufs=2)
            nc.vector.tensor_tensor(
                t2[:, :], rq[:, :], sin[:, c * QC:(c + 1) * QC], mybir.AluOpType.mult
            )
            nc.vector.tensor_tensor(out_slice, t1[:, :], t2[:, :], mybir.AluOpType.add)

        def k_proj_mms(c):
            """16 matmuls + psum->sbuf copy for chunk c's K projection.
            Returns the raw tile for the (separately emitted) rope."""
            xq = xq_tiles[c]
            ps = psum.tile([128, QC], F32, tag="pj", bufs=2)
            for i in range(HT):
                nc.tensor.matmul(
                    ps[:, :],
                    wk[:, i * 128:(i + 1) * 128],
                    xq[:, i * QC:(i + 1) * QC],
                    start=(i == 0),
                    stop=(i == HT - 1),
                )
            kraw = work.tile([128, QC], BF16, tag="raw", bufs=4)
            nc.scalar.mul(kraw[:, :], ps[:, :], 1.0)
            return kraw

        def v_proj_tile(c, t):
            """V projection for s-tile t of chunk c (natural [s, dh])."""
            xq = xq_tiles[c]
            st = c * TPC + t
            ps = psum.tile([128, 128], F32, tag="pj", bufs=2)
            for i in range(HT):
                nc.tensor.matmul(
                    ps[:, :],
                    xq[:, i * QC + t * 128:i * QC + (t + 1) * 128],
                    wv[:, i * 128:(i + 1) * 128],
                    start=(i == 0),
                    stop=(i == HT - 1),
                )
            nc.vector.tensor_copy(v_nat[:, st * 128:(st + 1) * 128], ps[:, :])

        def q_proj_head(c, j):
            xq = xq_tiles[c]
            ps = psum.tile([128, QC], F32, tag="pj", bufs=2)
            for i in range(HT):
                nc.tensor.matmul(
                    ps[:, :],
                    wq[:, (j * HT + i) * 128:(j * HT + i + 1) * 128],
                    xq[:, i * QC:(i + 1) * QC],
                    start=(i == 0),
                    stop=(i == HT - 1),
                )
            qraw = work.tile([128, QC], BF16, tag="raw", bufs=4)
            nc.scalar.mul(qraw[:, :], ps[:, :], 1.0)
            rope(qraw, q_ro[:, j * S + c * QC:j * S + (c + 1) * QC], c)

        # pending output-projection pieces of the previous chunk; drained as
        # PE filler between attention heads and Q-proj heads.
        pending = []
        yst_cur = [None]

        def oproj_piece():
            if not pending:
                return
            st, ho = pending.pop(0)
            yps = psum.tile([128, QC], F32, tag="pj", bufs=2)
            for j in range(NH):
                nc.tensor.matmul(
                    yps[:, :],
                    ot[:, j * S + st * 128:j * S + (st + 1) * 128],
                    wo[:, j * HID + ho * QC:j * HID + (ho + 1) * QC],
                    start=(j == 0),
                    stop=(j == NH - 1),
                )
            if ho == 0:
                yst_cur[0] = ysp.tile([128, HID], BF16, tag="yst", name="yst")
            yst = yst_cur[0]
            if ho % 2 == 0:
                nc.scalar.mul(yst[:, ho * QC:(ho + 1) * QC], yps[:, :], 1.0)
            else:
                nc.vector.tensor_copy(yst[:, ho * QC:(ho + 1) * QC], yps[:, :])
            if st >= S // 128 - TPC:
                # last chunk: one DMA per ho piece on rotating queues, so
                # the final writes drain as soon as each piece is ready
                eng = (nc.scalar, nc.gpsimd, nc.scalar, nc.sync)[ho]
                eng.dma_start(
                    d_y[st * 128:(st + 1) * 128, ho * QC:(ho + 1) * QC],
                    yst[:, ho * QC:(ho + 1) * QC],
                )
            elif ho == 1:
                nc.gpsimd.dma_start(
                    d_y[st * 128:(st + 1) * 128, :HID // 2], yst[:, :HID // 2]
                )
            elif ho == HID // QC - 1:
                nc.sync.dma_start(
                    d_y[st * 128:(st + 1) * 128, HID // 2:], yst[:, HID // 2:]
                )

        def push_chunk_pieces(cc):
            for t in range(TPC):
                for ho in range(HID // QC):
                    pending.append((cc * TPC + t, ho))

        deferred = [None]

        def attention_block(q_off, qw, fillers, pops, defer_last):
            """Attention for q columns [q_off, q_off+qw) of all local heads.
            fillers[j] = list of callables run after head j (K/V projections
            of the next chunk); pops[j] = oproj pieces popped after head j.
            defer_last: package the last head's finalize into deferred[]."""
            nfull = q_off // 128
            ntp = qw // 128
            for j in range(NH):
                q_base = j * S + q_off
                oacc = psum.tile([128, QC], F32, tag="acc", bufs=2)
                # softmax denominator: quartets of pt tiles are tree-summed
                # on DVE, then chained into a running bf16 sum, so a single
                # ones-matmul per (head, block) does the partition reduction
                # and the finalize chain stays short.
                dps = psum.tile([128, QC], F32, tag="dps", bufs=1)
                quartet = []
                den = [None]

                def fold_den(tiles, den=den, qw=qw):
                    """Combine 2 or 4 pt tiles (not written in place - AV
                    still reads them) into one new qsum tile, chained with
                    the running den via an in-place add."""
                    qs = work.tile([128, QC], BF16, tag="qsum", bufs=3)
                    if len(tiles) == 4:
                        q1 = work.tile([128, QC], BF16, tag="ptq", bufs=2)
                        nc.vector.tensor_tensor(
                            q1[:, :qw], tiles[0][:, :qw], tiles[1][:, :qw],
                            mybir.AluOpType.add,
                        )
                        nc.vector.tensor_tensor(
                            qs[:, :qw], tiles[2][:, :qw], tiles[3][:, :qw],
                            mybir.AluOpType.add,
                        )
                        nc.vector.tensor_tensor(
                            qs[:, :qw], qs[:, :qw], q1[:, :qw],
                            mybir.AluOpType.add,
                        )
                    else:
                        nc.vector.tensor_tensor(
                            qs[:, :qw], tiles[0][:, :qw], tiles[1][:, :qw],
                            mybir.AluOpType.add,
                        )
                    if den[0] is not None:
                        nc.vector.tensor_tensor(
                            qs[:, :qw], qs[:, :qw], den[0][:, :qw],
                            mybir.AluOpType.add,
                        )
                    den[0] = qs

                for kt in range(nfull):
                    sps = psum.tile([128, QC], F32, tag="sps", bufs=3)
                    nc.tensor.matmul(
                        sps[:, :qw],
                        k_ro[:, kt * 128:(kt + 1) * 128],
                        q_ro[:, q_base:q_base + qw],
                        start=True,
                        stop=True,
                    )
                    pt = ptp.tile([128, QC], BF16, tag="pt")
                    nc.scalar.activation(
                        pt[:, :qw], sps[:, :qw], mybir.ActivationFunctionType.Exp,
                        bias=0.0, scale=SCALE,
                    )
                    quartet.append(pt)
                    if len(quartet) == 4:
                        fold_den(quartet)
                        quartet = []
                    nc.tensor.matmul(
                        oacc[:, :qw],
                        v_nat[:, kt * 128:(kt + 1) * 128],
                        pt[:, :qw],
                        start=(kt == 0),
                        stop=False,
                    )
                if quartet:
                    fold_den(quartet)
                    quartet = []

                # diagonal qw x qw block, k-major: k-tile t covers only the
                # causally-reachable q columns [t*128, qw)
                ddn = work.tile([128, QC], BF16, tag="ddn", bufs=2)
                for t in range(ntp):
                    kt = nfull + t
                    w = qw - t * 128
                    spsd = psum.tile([128, QC], F32, tag="sps", bufs=3)
                    nc.tensor.matmul(
                        spsd[:, :w],
                        k_ro[:, kt * 128:(kt + 1) * 128],
                        q_ro[:, q_base + t * 128:q_base + qw],
                        start=True,
                        stop=True,
                    )
                    ptd = ptp.tile([128, QC], BF16, tag="pt")
                    nc.scalar.activation(
                        ptd[:, :w], spsd[:, :w], mybir.ActivationFunctionType.Exp,
                        bias=0.0, scale=SCALE,
                    )
                    # only the leading 128 columns straddle the diagonal
                    nc.vector.tensor_tensor(
                        ptd[:, :128], ptd[:, :128], tri[:, :], mybir.AluOpType.mult
                    )
                    if t == 0:
                        nc.vector.tensor_copy(ddn[:, :qw], ptd[:, :qw])
                    else:
                        nc.vector.tensor_tensor(
                            ddn[:, t * 128:qw], ddn[:, t * 128:qw], ptd[:, :w],
                            mybir.AluOpType.add,
                        )
                    # AV: first 128 cols end their accumulation group here
                    nc.tensor.matmul(
                        oacc[:, t * 128:(t + 1) * 128],
                        v_nat[:, kt * 128:(kt + 1) * 128],
                        ptd[:, :128],
                        start=(nfull == 0 and t == 0),
                        stop=True,
                    )
                    if w > 128:
                        nc.tensor.matmul(
                            oacc[:, (t + 1) * 128:qw],
                            v_nat[:, kt * 128:(kt + 1) * 128],
                            ptd[:, 128:w],
                            start=(nfull == 0 and t == 0),
                            stop=False,
                        )

                def finalize(oacc=oacc, dps=dps, ddn=ddn, q_base=q_base,
                             den=den, qw=qw):
                    if den[0] is not None:
                        nc.vector.tensor_tensor(
                            ddn[:, :qw], ddn[:, :qw], den[0][:, :qw],
                            mybir.AluOpType.add,
                        )
                    nc.tensor.matmul(dps[:, :qw], ones[:, :], ddn[:, :qw],
                                     start=True, stop=True)
                    rec = work.tile([128, QC], F32, tag="rec", bufs=2)
                    nc.vector.reciprocal_approx_fast(
                        out=rec[:, :qw], in_=dps[:, :qw])
                    nc.vector.tensor_tensor(
                        ot[:, q_base:q_base + qw],
                        oacc[:, :qw], rec[:, :qw], mybir.AluOpType.mult,
                    )
                if j == NH - 1 and defer_last:
                    deferred[0] = finalize  # run once the next phase starts
                else:
                    finalize()

                for f in fillers[j]:
                    f()
                if j == 0 and deferred[0] is not None:
                    deferred[0]()
                    deferred[0] = None
                for _ in range(pops[j]):
                    oproj_piece()

        # ---- chunk 0 startup: K/Q0/Q1 projections interleaved i-major in
        # one pass over the x hid-tiles (3 concurrent psum accumulation
        # groups), so the single DMA-paced pass covers 3 projections.
        psK = psum.tile([128, QC], F32, tag="pj", bufs=2)
        psQ0 = psum.tile([128, QC], F32, tag="acc", bufs=2)
        psQ1 = psum.tile([128, QC], F32, tag="dps", bufs=1)
        for i in range(HT):
            nc.tensor.matmul(
                psK[:, :], wk[:, i * 128:(i + 1) * 128],
                xq0[:, i * QC:(i + 1) * QC],
                start=(i == 0), stop=(i == HT - 1),
            )
            nc.tensor.matmul(
                psQ0[:, :], wq[:, i * 128:(i + 1) * 128],
                xq0[:, i * QC:(i + 1) * QC],
                start=(i == 0), stop=(i == HT - 1),
            )
            nc.tensor.matmul(
                psQ1[:, :], wq[:, (HT + i) * 128:(HT + i + 1) * 128],
                xq0[:, i * QC:(i + 1) * QC],
                start=(i == 0), stop=(i == HT - 1),
            )
        kraw0 = work.tile([128, QC], BF16, tag="raw", bufs=4)
        nc.scalar.mul(kraw0[:, :], psK[:, :], 1.0)
        q0raw = work.tile([128, QC], BF16, tag="raw", bufs=4)
        nc.scalar.mul(q0raw[:, :], psQ0[:, :], 1.0)
        q1raw = work.tile([128, QC], BF16, tag="raw", bufs=4)
        nc.scalar.mul(q1raw[:, :], psQ1[:, :], 1.0)
        rope(kraw0, k_ro[:, 0:QC], 0)
        rope(q0raw, q_ro[:, 0:QC], 0)
        rope(q1raw, q_ro[:, S:S + QC], 0)

        for c in range(SC):
            # ---- prefetch x for chunk c+1 (consumed by the K/V filler
            # inside this chunk's attention phase)
            if c + 1 < SC:
                xq_next = xtp.tile([128, HT * QC], BF16, tag="xq")
                xq_tiles[c + 1] = xq_next
                for i in range(4):
                    nc.sync.dma_start(
                        xq_next[:, i * W:(i + 1) * W],
                        d_xt[:, (c + 1) * HT * QC + i * W:
                             (c + 1) * HT * QC + (i + 1) * W],
                    )

            # ---- Q projections + rope for chunk c (heads 0-1 of chunk 0
            # were projected in the startup pass; its V tiles follow here),
            # with the previous chunk's deferred finalize + a few oproj
            # pieces as filler
            for j in range(NH):
                if c > 0 or j >= 2:
                    q_proj_head(c, j)
                elif j == 0:
                    v_proj_tile(0, 0)
                    v_proj_tile(0, 1)
                else:
                    v_proj_tile(0, 2)
                    v_proj_tile(0, 3)
                if j == 0 and deferred[0] is not None:
                    deferred[0]()
                    deferred[0] = None
                else:
                    oproj_piece()

            # ---- attention for q-chunk c. After each head: K/V projection
            # filler for chunk c+1, then oproj pieces of chunk c-1. The
            # last chunk runs as two half-width blocks so the final oproj
            # drain (which can't overlap anything) halves.
            kraw_next = [None]
            if c + 1 < SC:
                def fk(cc=c + 1):
                    kraw_next[0] = k_proj_mms(cc)
                def fr(cc=c + 1):
                    rope(kraw_next[0], k_ro[:, cc * QC:(cc + 1) * QC], cc)
                if c == 0:
                    # K1 already emitted after chunk 0's Q phase; spread the
                    # V tiles so every head transition has PE filler
                    fillers = [
                        [lambda: v_proj_tile(1, 0)],
                        [fr, lambda: v_proj_tile(1, 1)],
                        [lambda: v_proj_tile(1, 2)],
                        [lambda: v_proj_tile(1, 3)],
                    ]
                    fk()
                else:
                    fillers = [
                        [fk],
                        [fr, lambda cc=c + 1: v_proj_tile(cc, 0)],
                        [lambda cc=c + 1: v_proj_tile(cc, 1),
                         lambda cc=c + 1: v_proj_tile(cc, 2)],
                        [lambda cc=c + 1: v_proj_tile(cc, 3)],
                    ]
                attention_block(c * QC, QC, fillers, [4, 3, 3, 3],
                                defer_last=True)
            else:
                attention_block(c * QC, QC, [[], [], [], []], [4, 3, 3, 3],
                                defer_last=False)
            while pending:
                oproj_piece()
            push_chunk_pieces(c)

        while pending:  # drain: last chunk's output projection
            oproj_piece()

    if not nc.is_finalized():
        nc.finalize()
    return nc


def host_prep_x(x_b, S=S, HID=HID):
    """x [S, HID] -> device xT layout [128, SC*HT*QC] (bf16)."""
    HT = HID // 128
    SC = S // QC
    xx = x_b.astype(NP_BF16)
    # xt[p, ((c*HT)+i)*QC + s] = x[c*QC+s, i*128+p]
    return np.ascontiguousarray(
        xx.reshape(SC, QC, HT, 128).transpose(3, 0, 2, 1).reshape(128, SC * HT * QC)
    )


def host_prep_tables(pos0, S=S):
    """RoPE cos/sin tables, rotation matrix, diag tri mask, ones (shared)."""
    inv_freq = 1.0 / (BASE ** (np.arange(0, DH, 2, dtype=np.float32) / DH))
    freqs = pos0.astype(np.float32)[:, None] * inv_freq[None, :]  # [S, 64]
    emb = np.concatenate([freqs, freqs], axis=-1)  # [S, DH]
    cosT = np.ascontiguousarray(np.cos(emb).T.astype(NP_BF16))  # [128, S]
    sinT = np.ascontiguousarray(np.sin(emb).T.astype(NP_BF16))

    # R^T for interleaved rotate_half: rh = R @ q, R[2i,2i+1]=-1, R[2i+1,2i]=1
    R = np.zeros((DH, DH), dtype=np.float32)
    ii = np.arange(0, DH, 2)
    R[ii, ii + 1] = -1.0
    R[ii + 1, ii] = 1.0
    rT = np.ascontiguousarray(R.T.astype(NP_BF16))

    # 128x128 diagonal causal mask in [k, q] layout: tri[kk, qq] = qq >= kk
    kk = np.arange(128)[:, None]
    qq = np.arange(128)[None, :]
    tri = np.ascontiguousarray((qq >= kk).astype(NP_BF16))
    ones = np.ones((128, 128), dtype=NP_BF16)
    return {"cos": cosT, "sin": sinT, "tri": tri, "rt": rT, "ones": ones}


def host_prep_weights(Wq, Wk, Wv, Wo, g, HID=HID, NH=NH):
    """Per-kv-group weight shards in device layouts (bf16)."""
    HT = HID // 128

    wq_s = Wq[NH * 128 * g:NH * 128 * (g + 1), :].astype(NP_BF16)  # [NH*128, HID]
    # wq[p, i*128*NH + j*128 + d2] -> for lhsT [hid, dh]: value Wq_s[j*128+d2, i*128+p]
    wq = np.ascontiguousarray(
        wq_s.reshape(NH, 128, HT, 128).transpose(3, 0, 2, 1).reshape(128, NH * HT * 128)
    )
    # wq[p, (j*HT + i)*128 + d] = wq_s[j, d, i, p] = Wq_s[j*128+d, i*128+p]

    wk_s = Wk[128 * g:128 * (g + 1), :].astype(NP_BF16)  # [128, HID]
    wv_s = Wv[128 * g:128 * (g + 1), :].astype(NP_BF16)
    # wk[p, i*128 + u] = wk_s[u, i*128+p]
    wk = np.ascontiguousarray(
        wk_s.reshape(128, HT, 128).transpose(2, 1, 0).reshape(128, HT * 128)
    )
    wv = np.ascontiguousarray(
        wv_s.reshape(128, HT, 128).transpose(2, 1, 0).reshape(128, HT * 128)
    )

    wo_s = Wo[:, NH * 128 * g:NH * 128 * (g + 1)].astype(NP_BF16)  # [HID, NH*128]
    wo = np.ascontiguousarray(
        wo_s.reshape(HID, NH, 128).transpose(2, 1, 0).reshape(128, NH * HID)
    )
    # wo[p, j*HID + o] = wo_s[o, j*128+p] = Wo[o, cols0 + j*128+p]  OK

    return {"wq": wq, "wk": wk, "wv": wv, "wo": wo}


_NC_CACHE = {}


def kernel(x, position_ids, Wq, Wk, Wv, Wo):
    x = np.asarray(x, dtype=np.float32)
    position_ids = np.asarray(position_ids)
    Wq = np.asarray(Wq, dtype=np.float32)
    Wk = np.asarray(Wk, dtype=np.float32)
    Wv = np.asarray(Wv, dtype=np.float32)
    Wo = np.asarray(Wo, dtype=np.float32)
    assert x.shape == (B, S, HID), x.shape

    if "nc" not in _NC_CACHE:
        _NC_CACHE["nc"] = build_nc()
    nc = _NC_CACHE["nc"]

    pos0 = position_ids[0]  # reference uses row 0 for both batches
    tables = host_prep_tables(pos0)
    xts = [host_prep_x(x[b]) for b in range(B)]
    wshards = [host_prep_weights(Wq, Wk, Wv, Wo, g) for g in range(HK)]
    in_maps = []
    for c in range(N_CORES):
        b, g = divmod(c, HK)
        in_maps.append({"xt": xts[b], **wshards[g], **tables})

    trace = bool(int(os.environ.get("ATTN_TRACE", "0")))
    tmpdir = os.environ.get("ATTN_TRACE_DIR") or None
    if tmpdir is not None:
        LAST_RUN["n"] = LAST_RUN.get("n", 0) + 1
        tmpdir = os.path.join(tmpdir, f"run{LAST_RUN['n']}")
        os.makedirs(tmpdir, exist_ok=True)
    res = run_bass_kernel_spmd(
        nc, in_maps, list(range(N_CORES)), trace=trace, tmpdir=tmpdir
    )
    LAST_RUN["exec_time_ns"] = res.exec_time_ns
    LAST_RUN["mean_exec_time_ns"] = res.mean_exec_time_ns
    LAST_RUN["trace_dir"] = tmpdir

    out = np.zeros((B, S, HID), dtype=np.float32)
    for c in range(N_CORES):
        b = c // HK
        out[b] += res.results[c]["y"].astype(np.float32)
    return out



# revision 18
# speedup vs baseline: 1.0117x; 1.0117x over previous
"""Causal GQA self-attention (B=2, S=2048, HID=2048, 16 q heads / 4 kv heads,
DH=128, interleaved RoPE) as a Trainium2 Bass/Tile kernel on 8 NeuronCores.

Sharding: core c -> (batch b = c // 4, kv-group g = c % 4). Each core computes
its batch's attention for the 4 q heads served by kv head g, plus the partial
output projection against Wo[:, group cols]; the host sums the 4 partials per
batch.

Device dataflow (per core), everything in "transposed" [feature, seq] layout:
  xT   [hid, s]   (host-pretransposed, bf16)
  qT   = WqT.T @ xT          (per head, psum f32)  -> rope -> bf16
  kT   = WkT.T @ xT          -> rope -> bf16
  v    [s, dh]    = xT.T @ WvT   (natural layout, bf16)
  sT   [s_k, s_q] = kT_tile.T @ qT_chunk      (one MM per tile, causal skip;
        the diagonal 512x512 block runs k-major with shrinking free dim)
  pT   = exp(sT * scale) (* diag mask)        (ACT, bf16; softmax max-free)
  den  = elementwise-accumulated pT on Pool/DVE (f32 sbuf), then one
         ones-matmul per (head, chunk) broadcasts the partition reduction
  oT  += v_tile.T @ pT                         (psum f32 accum)
  OT   = oT * (1/den)    (bf16)
  y   += OT_tile.T @ WoT_chunk  over heads    (psum f32) -> DRAM f32
"""

import os
import sys
from contextlib import ExitStack

import numpy as np

sys.path.insert(0, "/opt/trn_rl_repo")

import ml_dtypes

import concourse.bass as bass
import concourse.mybir as mybir
import concourse.tile as tile
from concourse import bacc
from concourse.bass_utils import run_bass_kernel_spmd

BF16 = mybir.dt.bfloat16
F32 = mybir.dt.float32
NP_BF16 = ml_dtypes.bfloat16

# problem constants
B, S, HID = 2, 2048, 2048
H, HK, DH = 16, 4, 128
NH = H // HK  # local q heads per core (= REP)
BASE = 10000.0
SCALE = 1.0 / float(np.sqrt(DH))

N_CORES = 8
QC = 512            # q-chunk (psum free dim)
KT = 128            # k-tile (partition dim)

LAST_RUN = {}


def build_nc(S=S, HID=HID, NH=NH):
    """Build the per-core Bass program. All cores run the same program (SPMD).

    Software pipeline: chunk c's section = [deferred finalize of c-1's last
    head] -> Q proj c -> attention c. K/V projections for chunk c+1 are
    interleaved INTO attention c as PE filler (they only need xq_{c+1} +
    weights), so the PE never stalls on the exp/mask chains; output
    projection pieces of chunk c-1 fill the remaining slack.
    """
    HT = HID // 128        # hid tiles
    SC = S // QC           # s chunks
    ST = S // 128          # s tiles
    TPC = QC // 128        # 128-tiles per chunk

    nc = bacc.Bacc()

    d_xt = nc.declare_dram_parameter("xt", [128, SC * HT * QC], BF16, isOutput=False)
    d_wq = nc.declare_dram_parameter("wq", [128, HT * 128 * NH], BF16, isOutput=False)
    d_wk = nc.declare_dram_parameter("wk", [128, HT * 128], BF16, isOutput=False)
    d_wv = nc.declare_dram_parameter("wv", [128, HT * 128], BF16, isOutput=False)
    d_wo = nc.declare_dram_parameter("wo", [128, NH * HID], BF16, isOutput=False)
    d_cos = nc.declare_dram_parameter("cos", [128, S], BF16, isOutput=False)
    d_sin = nc.declare_dram_parameter("sin", [128, S], BF16, isOutput=False)
    d_tri = nc.declare_dram_parameter("tri", [128, 128], BF16, isOutput=False)
    d_rt = nc.declare_dram_parameter("rt", [128, 128], BF16, isOutput=False)
    d_ones = nc.declare_dram_parameter("ones", [128, 128], BF16, isOutput=False)
    d_y = nc.declare_dram_parameter("y", [S, HID], BF16, isOutput=True)

    with tile.TileContext(nc) as tc, ExitStack() as ctx:
        const = ctx.enter_context(tc.tile_pool(name="const", bufs=1))
        xtp = ctx.enter_context(tc.tile_pool(name="xtp", bufs=2))
        work = ctx.enter_context(tc.tile_pool(name="work", bufs=1))
        ptp = ctx.enter_context(tc.tile_pool(name="ptp", bufs=8))
        ysp = ctx.enter_context(tc.tile_pool(name="ysp", bufs=4))
        psum = ctx.enter_context(tc.tile_pool(name="psum", bufs=1, space="PSUM"))

        # --- PE warmup: tiny matmuls on a memset tile keep the PE busy (and
        # the HAM clock un-throttled) while the first input DMAs land.
        warm = const.tile([128, 128], BF16, tag="warm")
        nc.vector.memset(warm[:, :], 0.0)
        for _ in range(16):
            wps = psum.tile([128, QC], F32, tag="dps", bufs=1)
            nc.tensor.matmul(wps[:, :128], warm[:, :], warm[:, :],
                             start=True, stop=True)

        # --- persistent constants (DMA order = need order: K-proj inputs
        # first, Wo (used only by the output projection) last) ---
        xq0 = xtp.tile([128, HT * QC], BF16, tag="xq")
        NDMA = 4  # split the chunk load into 4 large DMAs (issue cost ~0.7us each)
        W = HT * QC // NDMA
        for i in range(NDMA):
            nc.sync.dma_start(
                xq0[:, i * W:(i + 1) * W], d_xt[:, i * W:(i + 1) * W]
            )
        wk = const.tile([128, HT * 128], BF16, tag="wk")
        nc.scalar.dma_start(wk[:, :], d_wk[:, :])
        wq = const.tile([128, HT * 128 * NH], BF16, tag="wq")
        for j in range(2):  # heads 0-1 feed the interleaved startup pass
            nc.scalar.dma_start(
                wq[:, j * HT * 128:(j + 1) * HT * 128],
                d_wq[:, j * HT * 128:(j + 1) * HT * 128],
            )
        cos = const.tile([128, S], BF16, tag="cos")
        sin = const.tile([128, S], BF16, tag="sin")
        nc.scalar.dma_start(cos[:, :QC], d_cos[:, :QC])
        nc.scalar.dma_start(sin[:, :QC], d_sin[:, :QC])
        rt = const.tile([128, 128], BF16, tag="rt")
        nc.scalar.dma_start(rt[:, :], d_rt[:, :])
        nc.scalar.dma_start(
            wq[:, 2 * HT * 128:3 * HT * 128], d_wq[:, 2 * HT * 128:3 * HT * 128]
        )
        wv = const.tile([128, HT * 128], BF16, tag="wv")
        nc.scalar.dma_start(wv[:, :], d_wv[:, :])
        nc.scalar.dma_start(
            wq[:, 3 * HT * 128:4 * HT * 128], d_wq[:, 3 * HT * 128:4 * HT * 128]
        )
        nc.scalar.dma_start(cos[:, QC:], d_cos[:, QC:])
        nc.scalar.dma_start(sin[:, QC:], d_sin[:, QC:])
        tri = const.tile([128, 128], BF16, tag="tri")
        nc.scalar.dma_start(tri[:, :], d_tri[:, :])
        ones = const.tile([128, 128], BF16, tag="ones")
        nc.scalar.dma_start(ones[:, :], d_ones[:, :])
        wo = const.tile([128, NH * HID], BF16, tag="wo")
        nc.scalar.dma_start(wo[:, :], d_wo[:, :])

        # persistent activations
        q_ro = const.tile([128, NH * S], BF16, tag="q_ro")
        k_ro = const.tile([128, S], BF16, tag="k_ro")
        v_nat = const.tile([128, S], BF16, tag="v_nat")
        ot = const.tile([128, NH * S], BF16, tag="ot")

        xq_tiles = {0: xq0}

        def rope(raw, out_slice, c):
            """raw: [128, QC] bf16 sbuf tile (pre-rope head block, dh on
            partitions). Writes bf16 roped output to out_slice."""
            rq = psum.tile([128, QC], F32, tag="sps", bufs=3)
            nc.tensor.matmul(rq[:, :], rt[:, :], raw[:, :], start=True, stop=True)
            t1 = work.tile([128, QC], BF16, tag="t1", bufs=2)
            nc.vector.tensor_tensor(
                t1[:, :], raw[:, :], cos[:, c * QC:(c + 1) * QC], mybir.AluOpType.mult
            )
            t2 = work.tile([128, QC], BF16, tag="t2", bufs=2)
            nc.vector.tensor_tensor(
                t2[:, :], rq[:, :], sin[:, c * QC:(c + 1) * QC], mybir.AluOpType.mult
            )
            nc.vector.tensor_tensor(out_slice, t1[:, :], t2[:, :], mybir.AluOpType.add)

        def k_proj_mms(c):
            """16 matmuls + psum->sbuf copy for chunk c's K projection.
            Returns the raw tile for the (separately emitted) rope."""
            xq = xq_tiles[c]
            ps = psum.tile([128, QC], F32, tag="pj", bufs=2)
            for i in range(HT):
                nc.tensor.matmul(
                    ps[:, :],
                    wk[:, i * 128:(i + 1) * 128],
                    xq[:, i * QC:(i + 1) * QC],
                    start=(i == 0),
                    stop=(i == HT - 1),
                )
            kraw = work.tile([128, QC], BF16, tag="raw", bufs=4)
            nc.vector.tensor_copy(kraw[:, :], ps[:, :])
            return kraw

        def v_proj_tile(c, t):
            """V projection for s-tile t of chunk c (natural [s, dh])."""
            xq = xq_tiles[c]
            st = c * TPC + t
            ps = psum.tile([128, 128], F32, tag="pj", bufs=2)
            for i in range(HT):
                nc.tensor.matmul(
                    ps[:, :],
                    xq[:, i * QC + t * 128:i * QC + (t + 1) * 128],
                    wv[:, i * 128:(i + 1) * 128],
                    start=(i == 0),
                    stop=(i == HT - 1),
                )
            nc.vector.tensor_copy(v_nat[:, st * 128:(st + 1) * 128], ps[:, :])

        def q_proj_head(c, j):
            xq = xq_tiles[c]
            ps = psum.tile([128, QC], F32, tag="pj", bufs=2)
            for i in range(HT):
                nc.tensor.matmul(
                    ps[:, :],
                    wq[:, (j * HT + i) * 128:(j * HT + i + 1) * 128],
                    xq[:, i * QC:(i + 1) * QC],
                    start=(i == 0),
                    stop=(i == HT - 1),
                )
            qraw = work.tile([128, QC], BF16, tag="raw", bufs=4)
            nc.scalar.mul(qraw[:, :], ps[:, :], 1.0)
            rope(qraw, q_ro[:, j * S + c * QC:j * S + (c + 1) * QC], c)

        # pending output-projection pieces of the previous chunk; drained as
        # PE filler between attention heads and Q-proj heads.
        pending = []
        yst_cur = [None]

        def oproj_piece():
            if not pending:
                return
            st, ho = pending.pop(0)
            yps = psum.tile([128, QC], F32, tag="pj", bufs=2)
            for j in range(NH):
                nc.tensor.matmul(
                    yps[:, :],
                    ot[:, j * S + st * 128:j * S + (st + 1) * 128],
                    wo[:, j * HID + ho * QC:j * HID + (ho + 1) * QC],
                    start=(j == 0),
                    stop=(j == NH - 1),
                )
            if ho == 0:
                yst_cur[0] = ysp.tile([128, HID], BF16, tag="yst", name="yst")
            yst = yst_cur[0]
            if ho == 2:
                nc.scalar.mul(yst[:, ho * QC:(ho + 1) * QC], yps[:, :], 1.0)
            else:
                nc.vector.tensor_copy(yst[:, ho * QC:(ho + 1) * QC], yps[:, :])
            if st >= S // 128 - TPC:
                # last chunk: one DMA per ho piece on rotating queues, so
                # the final writes drain as soon as each piece is ready
                eng = (nc.gpsimd, nc.gpsimd, nc.sync, nc.sync)[ho]
                eng.dma_start(
                    d_y[st * 128:(st + 1) * 128, ho * QC:(ho + 1) * QC],
                    yst[:, ho * QC:(ho + 1) * QC],
                )
            elif ho == 1:
                nc.gpsimd.dma_start(
                    d_y[st * 128:(st + 1) * 128, :HID // 2], yst[:, :HID // 2]
                )
            elif ho == HID // QC - 1:
                nc.sync.dma_start(
                    d_y[st * 128:(st + 1) * 128, HID // 2:], yst[:, HID // 2:]
                )

        def push_chunk_pieces(cc):
            for t in range(TPC):
                for ho in range(HID // QC):
                    pending.append((cc * TPC + t, ho))

        deferred = [None]

        def attention_block(q_off, qw, fillers, pops, defer_last):
            """Attention for q columns [q_off, q_off+qw) of all local heads.
            fillers[j] = list of callables run after head j (K/V projections
            of the next chunk); pops[j] = oproj pieces popped after head j.
            defer_last: package the last head's finalize into deferred[]."""
            nfull = q_off // 128
            ntp = qw // 128
            for j in range(NH):
                q_base = j * S + q_off
                oacc = psum.tile([128, QC], F32, tag="acc", bufs=2)
                # softmax denominator: quartets of pt tiles are tree-summed
                # on DVE, then chained into a running bf16 sum, so a single
                # ones-matmul per (head, block) does the partition reduction
                # and the finalize chain stays short.
                dps = psum.tile([128, QC], F32, tag="dps", bufs=1)
                quartet = []
                den = [None]

                def fold_den(tiles, den=den, qw=qw):
                    """Combine 2 or 4 pt tiles (not written in place - AV
                    still reads them) into one new qsum tile, chained with
                    the running den via an in-place add."""
                    qs = work.tile([128, QC], BF16, tag="qsum", bufs=3)
                    if len(tiles) == 4:
                        q1 = work.tile([128, QC], BF16, tag="ptq", bufs=2)
                        nc.vector.tensor_tensor(
                            q1[:, :qw], tiles[0][:, :qw], tiles[1][:, :qw],
                            mybir.AluOpType.add,
                        )
                        nc.vector.tensor_tensor(
                            qs[:, :qw], tiles[2][:, :qw], tiles[3][:, :qw],
                            mybir.AluOpType.add,
                        )
                        nc.vector.tensor_tensor(
                            qs[:, :qw], qs[:, :qw], q1[:, :qw],
                            mybir.AluOpType.add,
                        )
                    else:
                        nc.vector.tensor_tensor(
                            qs[:, :qw], tiles[0][:, :qw], tiles[1][:, :qw],
                            mybir.AluOpType.add,
                        )
                    if den[0] is not None:
                        nc.vector.tensor_tensor(
                            qs[:, :qw], qs[:, :qw], den[0][:, :qw],
                            mybir.AluOpType.add,
                        )
                    den[0] = qs

                for kt in range(nfull):
                    sps = psum.tile([128, QC], F32, tag="sps", bufs=3)
                    nc.tensor.matmul(
                        sps[:, :qw],
                        k_ro[:, kt * 128:(kt + 1) * 128],
                        q_ro[:, q_base:q_base + qw],
                        start=True,
                        stop=True,
                    )
                    pt = ptp.tile([128, QC], BF16, tag="pt")
                    nc.scalar.activation(
                        pt[:, :qw], sps[:, :qw], mybir.ActivationFunctionType.Exp,
                        bias=0.0, scale=SCALE,
                    )
                    quartet.append(pt)
                    if len(quartet) == 4:
                        fold_den(quartet)
                        quartet = []
                    nc.tensor.matmul(
                        oacc[:, :qw],
                        v_nat[:, kt * 128:(kt + 1) * 128],
                        pt[:, :qw],
                        start=(kt == 0),
                        stop=False,
                    )
                if quartet:
                    fold_den(quartet)
                    quartet = []

                # diagonal qw x qw block, k-major: k-tile t covers only the
                # causally-reachable q columns [t*128, qw)
                ddn = work.tile([128, QC], BF16, tag="ddn", bufs=2)
                for t in range(ntp):
                    kt = nfull + t
                    w = qw - t * 128
                    spsd = psum.tile([128, QC], F32, tag="sps", bufs=3)
                    nc.tensor.matmul(
                        spsd[:, :w],
                        k_ro[:, kt * 128:(kt + 1) * 128],
                        q_ro[:, q_base + t * 128:q_base + qw],
                        start=True,
                        stop=True,
                    )
                    ptd = ptp.tile([128, QC], BF16, tag="pt")
                    nc.scalar.activation(
                        ptd[:, :w], spsd[:, :w], mybir.ActivationFunctionType.Exp,
                        bias=0.0, scale=SCALE,
                    )
                    # only the leading 128 columns straddle the diagonal
                    nc.vector.tensor_tensor(
                        ptd[:, :128], ptd[:, :128], tri[:, :], mybir.AluOpType.mult
                    )
                    if t == 0:
                        nc.vector.tensor_copy(ddn[:, :qw], ptd[:, :qw])
                    else:
                        nc.vector.tensor_tensor(
                            ddn[:, t * 128:qw], ddn[:, t * 128:qw], ptd[:, :w],
                            mybir.AluOpType.add,
                        )
                    # AV: first 128 cols end their accumulation group here
                    nc.tensor.matmul(
                        oacc[:, t * 128:(t + 1) * 128],
                        v_nat[:, kt * 128:(kt + 1) * 128],
                        ptd[:, :128],
                        start=(nfull == 0 and t == 0),
                        stop=True,
                    )
                    if w > 128:
                        nc.tensor.matmul(
                            oacc[:, (t + 1) * 128:qw],
                            v_nat[:, kt * 128:(kt + 1) * 128],
                            ptd[:, 128:w],
                            start=(nfull == 0 and t == 0),
                            stop=False,
                        )

                def finalize(oacc=oacc, dps=dps, ddn=ddn, q_base=q_base,
                             den=den, qw=qw):
                    if den[0] is not None:
                        nc.vector.tensor_tensor(
                            ddn[:, :qw], ddn[:, :qw], den[0][:, :qw],
                            mybir.AluOpType.add,
                        )
                    nc.tensor.matmul(dps[:, :qw], ones[:, :], ddn[:, :qw],
                                     start=True, stop=True)
                    rec = work.tile([128, QC], F32, tag="rec", bufs=2)
                    nc.vector.reciprocal_approx_fast(
                        out=rec[:, :qw], in_=dps[:, :qw])
                    nc.vector.tensor_tensor(
                        ot[:, q_base:q_base + qw],
                        oacc[:, :qw], rec[:, :qw], mybir.AluOpType.mult,
                    )
                if j == NH - 1 and defer_last:
                    deferred[0] = finalize  # run once the next phase starts
                else:
                    finalize()

                for f in fillers[j]:
                    f()
                if j == 0 and deferred[0] is not None:
                    deferred[0]()
                    deferred[0] = None
                for _ in range(pops[j]):
                    oproj_piece()

        # ---- chunk 0 startup: K/Q0/Q1 projections interleaved i-major in
        # one pass over the x hid-tiles (3 concurrent psum accumulation
        # groups), so the single DMA-paced pass covers 3 projections.
        psK = psum.tile([128, QC], F32, tag="pj", bufs=2)
        psQ0 = psum.tile([128, QC], F32, tag="acc", bufs=2)
        psQ1 = psum.tile([128, QC], F32, tag="dps", bufs=1)
        for i in range(HT):
            nc.tensor.matmul(
                psK[:, :], wk[:, i * 128:(i + 1) * 128],
                xq0[:, i * QC:(i + 1) * QC],
                start=(i == 0), stop=(i == HT - 1),
            )
            nc.tensor.matmul(
                psQ0[:, :], wq[:, i * 128:(i + 1) * 128],
                xq0[:, i * QC:(i + 1) * QC],
                start=(i == 0), stop=(i == HT - 1),
            )
            nc.tensor.matmul(
                psQ1[:, :], wq[:, (HT + i) * 128:(HT + i + 1) * 128],
                xq0[:, i * QC:(i + 1) * QC],
                start=(i == 0), stop=(i == HT - 1),
            )
        kraw0 = work.tile([128, QC], BF16, tag="raw", bufs=4)
        nc.scalar.mul(kraw0[:, :], psK[:, :], 1.0)
        q0raw = work.tile([128, QC], BF16, tag="raw", bufs=4)
        nc.scalar.mul(q0raw[:, :], psQ0[:, :], 1.0)
        q1raw = work.tile([128, QC], BF16, tag="raw", bufs=4)
        nc.scalar.mul(q1raw[:, :], psQ1[:, :], 1.0)
        rope(kraw0, k_ro[:, 0:QC], 0)
        rope(q0raw, q_ro[:, 0:QC], 0)
        rope(q1raw, q_ro[:, S:S + QC], 0)

        for c in range(SC):
            # ---- prefetch x for chunk c+1 (consumed by the K/V filler
            # inside this chunk's attention phase)
            if c + 1 < SC:
                xq_next = xtp.tile([128, HT * QC], BF16, tag="xq")
                xq_tiles[c + 1] = xq_next
                for i in range(4):
                    nc.sync.dma_start(
                        xq_next[:, i * W:(i + 1) * W],
                        d_xt[:, (c + 1) * HT * QC + i * W:
                             (c + 1) * HT * QC + (i + 1) * W],
                    )

            # ---- Q projections + rope for chunk c (heads 0-1 of chunk 0
            # were projected in the startup pass; its V tiles follow here),
            # with the previous chunk's deferred finalize + a few oproj
            # pieces as filler
            for j in range(NH):
                if c > 0:
                    q_proj_head(c, j)
                elif j == 0:
                    q_proj_head(0, 2)
                elif j == 1:
                    v_proj_tile(0, 0)
                    v_proj_tile(0, 1)
                elif j == 2:
                    q_proj_head(0, 3)
                else:
                    v_proj_tile(0, 2)
                    v_proj_tile(0, 3)
                if j == 0 and deferred[0] is not None:
                    deferred[0]()
                    deferred[0] = None
                else:
                    oproj_piece()

            # ---- attention for q-chunk c. After each head: K/V projection
            # filler for chunk c+1, then oproj pieces of chunk c-1. The
            # last chunk runs as two half-width blocks so the final oproj
            # drain (which can't overlap anything) halves.
            kraw_next = [None]
            if c + 1 < SC:
                def fk(cc=c + 1):
                    kraw_next[0] = k_proj_mms(cc)
                def fr(cc=c + 1):
                    rope(kraw_next[0], k_ro[:, cc * QC:(cc + 1) * QC], cc)
                if c == 0:
                    # K1 is emitted right after chunk 0's Q phase; spread
                    # the V tiles so every head transition has PE filler
                    fillers = [
                        [lambda: v_proj_tile(1, 0)],
                        [fr, lambda: v_proj_tile(1, 1)],
                        [lambda: v_proj_tile(1, 2)],
                        [lambda: v_proj_tile(1, 3)],
                    ]
                    fk()
                else:
                    fillers = [
                        [fk],
                        [fr, lambda cc=c + 1: v_proj_tile(cc, 0)],
                        [lambda cc=c + 1: v_proj_tile(cc, 1),
                         lambda cc=c + 1: v_proj_tile(cc, 2)],
                        [lambda cc=c + 1: v_proj_tile(cc, 3)],
                    ]
                attention_block(c * QC, QC, fillers, [4, 3, 3, 3],
                                defer_last=True)
            else:
                attention_block(c * QC, QC, [[], [], [], []], [4, 3, 3, 3],
                                defer_last=False)
            while pending:
                oproj_piece()
            push_chunk_pieces(c)

        while pending:  # drain: last chunk's output projection
            oproj_piece()

    if not nc.is_finalized():
        nc.finalize()
    return nc


def host_prep_x(x_b, S=S, HID=HID):
    """x [S, HID] -> device xT layout [128, SC*HT*QC] (bf16)."""
    HT = HID // 128
    SC = S // QC
    xx = x_b.astype(NP_BF16)
    # xt[p, ((c*HT)+i)*QC + s] = x[c*QC+s, i*128+p]
    return np.ascontiguousarray(
        xx.reshape(SC, QC, HT, 128).transpose(3, 0, 2, 1).reshape(128, SC * HT * QC)
    )


def host_prep_tables(pos0, S=S):
    """RoPE cos/sin tables, rotation matrix, diag tri mask, ones (shared)."""
    inv_freq = 1.0 / (BASE ** (np.arange(0, DH, 2, dtype=np.float32) / DH))
    freqs = pos0.astype(np.float32)[:, None] * inv_freq[None, :]  # [S, 64]
    emb = np.concatenate([freqs, freqs], axis=-1)  # [S, DH]
    cosT = np.ascontiguousarray(np.cos(emb).T.astype(NP_BF16))  # [128, S]
    sinT = np.ascontiguousarray(np.sin(emb).T.astype(NP_BF16))

    # R^T for interleaved rotate_half: rh = R @ q, R[2i,2i+1]=-1, R[2i+1,2i]=1
    R = np.zeros((DH, DH), dtype=np.float32)
    ii = np.arange(0, DH, 2)
    R[ii, ii + 1] = -1.0
    R[ii + 1, ii] = 1.0
    rT = np.ascontiguousarray(R.T.astype(NP_BF16))

    # 128x128 diagonal causal mask in [k, q] layout: tri[kk, qq] = qq >= kk
    kk = np.arange(128)[:, None]
    qq = np.arange(128)[None, :]
    tri = np.ascontiguousarray((qq >= kk).astype(NP_BF16))
    ones = np.ones((128, 128), dtype=NP_BF16)
    return {"cos": cosT, "sin": sinT, "tri": tri, "rt": rT, "ones": ones}


def host_prep_weights(Wq, Wk, Wv, Wo, g, HID=HID, NH=NH):
    """Per-kv-group weight shards in device layouts (bf16)."""
    HT = HID // 128

    wq_s = Wq[NH * 128 * g:NH * 128 * (g + 1), :].astype(NP_BF16)  # [NH*128, HID]
    # wq[p, i*128*NH + j*128 + d2] -> for lhsT [hid, dh]: value Wq_s[j*128+d2, i*128+p]
    wq = np.ascontiguousarray(
        wq_s.reshape(NH, 128, HT, 128).transpose(3, 0, 2, 1).reshape(128, NH * HT * 128)
    )
    # wq[p, (j*HT + i)*128 + d] = wq_s[j, d, i, p] = Wq_s[j*128+d, i*128+p]

    wk_s = Wk[128 * g:128 * (g + 1), :].astype(NP_BF16)  # [128, HID]
    wv_s = Wv[128 * g:128 * (g + 1), :].astype(NP_BF16)
    # wk[p, i*128 + u] = wk_s[u, i*128+p]
    wk = np.ascontiguousarray(
        wk_s.reshape(128, HT, 128).transpose(2, 1, 0).reshape(128, HT * 128)
    )
    wv = np.ascontiguousarray(
        wv_s.reshape(128, HT, 128).transpose(2, 1, 0).reshape(128, HT * 128)
    )

    wo_s = Wo[:, NH * 128 * g:NH * 128 * (g + 1)].astype(NP_BF16)  # [HID, NH*128]
    wo = np.ascontiguousarray(
        wo_s.reshape(HID, NH, 128).transpose(2, 1, 0).reshape(128, NH * HID)
    )
    # wo[p, j*HID + o] = wo_s[o, j*128+p] = Wo[o, cols0 + j*128+p]  OK

    return {"wq": wq, "wk": wk, "wv": wv, "wo": wo}


_NC_CACHE = {}


def kernel(x, position_ids, Wq, Wk, Wv, Wo):
    x = np.asarray(x, dtype=np.float32)
    position_ids = np.asarray(position_ids)
    Wq = np.asarray(Wq, dtype=np.float32)
    Wk = np.asarray(Wk, dtype=np.float32)
    Wv = np.asarray(Wv, dtype=np.float32)
    Wo = np.asarray(Wo, dtype=np.float32)
    assert x.shape == (B, S, HID), x.shape

    if "nc" not in _NC_CACHE:
        _NC_CACHE["nc"] = build_nc()
    nc = _NC_CACHE["nc"]

    pos0 = position_ids[0]  # reference uses row 0 for both batches
    tables = host_prep_tables(pos0)
    xts = [host_prep_x(x[b]) for b in range(B)]
    wshards = [host_prep_weights(Wq, Wk, Wv, Wo, g) for g in range(HK)]
    in_maps = []
    for c in range(N_CORES):
        b, g = divmod(c, HK)
        in_maps.append({"xt": xts[b], **wshards[g], **tables})

    trace = bool(int(os.environ.get("ATTN_TRACE", "0")))
    tmpdir = os.environ.get("ATTN_TRACE_DIR") or None
    if tmpdir is not None:
        LAST_RUN["n"] = LAST_RUN.get("n", 0) + 1
        tmpdir = os.path.join(tmpdir, f"run{LAST_RUN['n']}")
        os.makedirs(tmpdir, exist_ok=True)
    res = run_bass_kernel_spmd(
        nc, in_maps, list(range(N_CORES)), trace=trace, tmpdir=tmpdir
    )
    LAST_RUN["exec_time_ns"] = res.exec_time_ns
    LAST_RUN["mean_exec_time_ns"] = res.mean_exec_time_ns
    LAST_RUN["trace_dir"] = tmpdir

    out = np.zeros((B, S, HID), dtype=np.float32)
    for c in range(N_CORES):
        b = c // HK
        out[b] += res.results[c]["y"].astype(np.float32)
    return out

